# revision 22
# baseline (speedup 1.0000x reference)
"""AttentionPool Trainium2 Bass kernel (v5: mixed-precision fp8 DoubleRow).

Computes, for h:[N,512] f32, sorted batch_vec:[N] int, gate-MLP weights
W1/b1/W2/b2:
    gate  = gelu(h @ W1 + b1) @ W2 + b2            (erf gelu)
    alpha = segment_softmax(gate, batch_vec)       (1024 segments)
    out   = segment_sum(alpha[:,None] * h)         -> [1024, 512]

Sharding: data-parallel over graphs. Core c owns graphs [128c, 128c+128)
and the contiguous node range covering them (batch_vec sorted => segments
never straddle cores).

Structure ("flip" mode, requires b1 == 0 which holds for this module):
  mm1 computes z in [node, dout] layout per 128-node tile:
      z_t = (hT_k)^T @ W1_k  accumulated over k-chunks  -> psum [128n, 512d]
  On this part the measured per-matmul cost is rows x ~0.55ns (effective
  ~1.8GHz row rate; LDWEIGHTS hides under >=128-row streams), so the only
  lever is reducing streamed rows. mm1 rows are cut ~40% by a
  mixed-precision dout split (see NA below): douts are host-permuted by
  |W2| ascending; the low-|W2| "zone A" runs with both operands in fp8e4
  via k-paired DoubleRow matmuls (2 instrs of K=256 each instead of 4 of
  K=128, 2 fp8 MACs/cell/cycle), the high-|W2| "zone C" stays f16-rate
  (lhsT in f8e3 to halve its DMA). Gate error from fp8 scales with each
  zone's share of sum(W2^2); measured end-to-end rel err 1.67e-2 vs the
  2e-2 gate (f16 baseline was 4.6e-4).
  gelu on ACT -> a1 [128n, 512d] f16 (ACT scale=1/16 undoes the x16
  applied to W1 before quantization so fp8/f16 W1 values sit in normal
  range), then the gate dot-product runs on the DVE as a fused
  multiply+reduce against a partition-replicated W2:
      gate[n] = reduce_add(a1[n,:] * W2rep[n,:]) + b2
  exp runs in a few batches on ACT (table switches cost 1.3us); pool
  matmuls + the hp DMA stream trickle through the tensor stream between
  supertiles once their exp batch is done. DMA layouts are host-packed so
  every DMA reads 2-4KB contiguous per partition. The softmax
  max-subtraction is skipped: gates are O(1) so exp is safe in fp32, and
  the result is mathematically identical. Trailing all-padding 128-node
  tiles (from the 512-padded supertile grid) are skipped via t_used.
"""

import os
from contextlib import ExitStack, nullcontext

import numpy as np

import concourse.bass as bass
import concourse.mybir as mybir
from concourse import bacc
import concourse.tile as tile
from concourse.bass_utils import run_bass_kernel_spmd

F32 = mybir.dt.float32
F16 = mybir.dt.float16
F8E3 = mybir.dt.float8e3
F8E4 = mybir.dt.float8e4

N_NODES = 100000
H = 512
NUM_GRAPHS = 1024
N_CORES = 8
G = NUM_GRAPHS // N_CORES  # graphs per core = 128
NP_DEFAULT = 12800         # padded nodes per core (25 supertiles of 512)

MODE = os.environ.get("AP_MODE", "flip")   # "flip" | "mm2"
HP_BUFS = int(os.environ.get("AP_HP_BUFS", "12"))
HT_BUFS = int(os.environ.get("AP_HT_BUFS", "4"))
# supertile indices after which each exp batch runs (must end with S-1).
# Two batches: with UNROLL=2 the pool tail hides under the next iteration's
# mm1 head, so a third batch only adds ACT table switches (1.3us each).
EXP_AT = tuple(int(x) for x in os.environ.get("AP_EXP_AT", "8,16,24").split(","))
# max pool supertiles trickled into the tensor stream per phase-A step
POOL_RATE = int(os.environ.get("AP_POOL_RATE", "3"))
# engine queue for the hp DMA stream ("sync" shares the hx queue)
HPQ = os.environ.get("AP_HPQ", "sync")
# mm1 rhs stream width (diagnostic: 256 doubles the weight-load count)
MM1_STREAM = int(os.environ.get("AP_MM1_STREAM", "512"))
# denominator path: "mm" (PE matmuls) | "gps" (GPSIMD partition reduce)
DENOM = os.environ.get("AP_DENOM", "mm")
# hx (gate-path h) dtype: f16 | f8e3 (e3m4, halves hx DMA + lhsT load bytes)
HXDT = os.environ.get("AP_HXDT", "f8e3")
# psz pool bufs override (0 = per-mode default)
PSZ = int(os.environ.get("AP_PSZ", "0"))
# loop-body unroll factor inside the reps hardware loop: 2 lets iteration
# N+1's mm1 stream start under iteration N's exp/pool/divide tail (~6us/rep);
# 4 regresses (program size). reps not divisible by it fall back to 1.
UNROLL = int(os.environ.get("AP_UNROLL", "2"))
# mm1 mixed-precision split (flip mode): douts are host-permuted by |W2|
# ascending; the first NA ("zone A") are computed with both operands in
# fp8e4 via k-paired DoubleRow matmuls (2 instead of 4 PE instructions,
# each streaming 2 fp8 MACs/cell/cycle), the rest ("zone C") in f16. Gate
# error from zone A scales with the zone's share of sum(W2^2), which the
# permutation makes small. W1 is scaled x16 before quantization so its
# e4m3/f16 values sit in normal range; gelu un-scales via ACT `scale`.
# NA=320: sim rel-err 1.2e-2 (gate 2e-2), mm1 113->81us. NA=0 disables.
NA = int(os.environ.get("AP_NA", "320"))
W1SC = 16.0


def _build(np_pad: int, mode: str = None, reps: int = 1, ablate: str = "",
           t_used: int = 0):
    """Build the per-core Bass program (SPMD: same program, per-core data).

    t_used: number of 128-node tiles that can hold real nodes (from the
    actual max per-core count); trailing all-padding tiles are skipped
    entirely (no mm1/gelu/gate/pool work). 0 = all tiles.
    """
    if mode is None:
        mode = MODE
    T = np_pad // 128          # 128-node tiles
    TU = t_used or T           # tiles with real work
    S = np_pad // 512          # 512-node supertiles
    KC = H // 128              # contraction chunks = 4

    nc = bacc.Bacc("TRN2", target_bir_lowering=False, debug=False)

    hx_dt = F8E3 if HXDT == "f8e3" else F16
    # [S, p, k, n] - 4KB (f16) or 2KB (fp8) per partition per supertile
    hx_d = nc.dram_tensor("hx", [S, 128, KC, 512], hx_dt, kind="ExternalInput")
    w1x_d = nc.dram_tensor("w1x", [KC, 128, 512], F16, kind="ExternalInput")
    hp_d = nc.dram_tensor("hp", [S, 128, 4, 512], F16, kind="ExternalInput")
    b2_d = nc.dram_tensor("b2t", [128, 1], F32, kind="ExternalInput")
    bv_d = nc.dram_tensor("bvrel", [128, T], F32, kind="ExternalInput")
    io_d = nc.dram_tensor("iota", [128, 128], F32, kind="ExternalInput")
    if mode == "flip" and (NA > 0 or ablate == "dr8"):
        hx8_d = nc.dram_tensor("hx8", [S, 128, KC, 512], F8E4,
                               kind="ExternalInput")
    if ablate == "dr8":
        w18_d = nc.dram_tensor("w1x8", [128, KC, 512], F8E4,
                               kind="ExternalInput")
    if mode == "flip" and NA > 0:
        w1a8_d = nc.dram_tensor("w1a8", [128, KC, NA], F8E4,
                                kind="ExternalInput")
    if mode == "flip":
        w2r_d = nc.dram_tensor("w2rep", [128, H], F16, kind="ExternalInput")
    else:
        b1_d = nc.dram_tensor("b1v", [128, KC], F32, kind="ExternalInput")
        w2_d = nc.dram_tensor("W2v", [128, KC * 2], F16, kind="ExternalInput")
    out_d = nc.dram_tensor("out", [G, H], F32, kind="ExternalOutput")

    with tile.TileContext(nc) as tc, ExitStack() as ctx:
        consts = ctx.enter_context(tc.tile_pool(name="consts", bufs=1))
        ht_pool = ctx.enter_context(tc.tile_pool(name="ht", bufs=HT_BUFS))
        a1_pool = ctx.enter_context(tc.tile_pool(name="a1", bufs=8))
        hp_pool = ctx.enter_context(tc.tile_pool(name="hp", bufs=HP_BUFS))
        ms_pool = ctx.enter_context(tc.tile_pool(name="ms", bufs=4))
        small = ctx.enter_context(tc.tile_pool(name="small", bufs=2))
        nzb = PSZ or (6 if mode == "flip" else 4)
        psz = ctx.enter_context(tc.tile_pool(name="psz", bufs=nzb, space="PSUM"))
        if mode != "flip":
            psg = ctx.enter_context(tc.tile_pool(name="psg", bufs=2,
                                                 space="PSUM"))
        psp = ctx.enter_context(tc.tile_pool(name="psp", bufs=1, space="PSUM"))
        psd = ctx.enter_context(tc.tile_pool(name="psd", bufs=1, space="PSUM"))

        # ---- constants ----
        w1_sb = []
        for k in range(KC):
            t = consts.tile([128, 512], F16, tag=f"w1_{k}")
            nc.sync.dma_start(out=t, in_=w1x_d.ap()[k])
            w1_sb.append(t)
        if mode == "flip":
            w2r_sb = consts.tile([128, H], F16, tag="w2rep")
            nc.sync.dma_start(out=w2r_sb, in_=w2r_d.ap())
            gs_pool = ctx.enter_context(tc.tile_pool(name="gs", bufs=2))
            if NA > 0:
                w1a8_sb = consts.tile([128, KC, NA], F8E4, tag="w1a8")
                nc.sync.dma_start(out=w1a8_sb, in_=w1a8_d.ap())
                ht8_pool = ctx.enter_context(
                    tc.tile_pool(name="ht8", bufs=HT_BUFS))
        else:
            b1_sb = consts.tile([128, KC], F32, tag="b1")
            nc.sync.dma_start(out=b1_sb, in_=b1_d.ap())
            w2_sb = consts.tile([128, KC * 2], F16, tag="w2")
            nc.sync.dma_start(out=w2_sb, in_=w2_d.ap())
        b2_sb = consts.tile([128, 1], F32, tag="b2")
        nc.sync.dma_start(out=b2_sb, in_=b2_d.ap())
        io_sb = consts.tile([128, 128], F32, tag="iota")
        nc.sync.dma_start(out=io_sb, in_=io_d.ap())
        bv_sb = consts.tile([128, T], F32, tag="bv")
        nc.sync.dma_start(out=bv_sb, in_=bv_d.ap())
        ones_sb = consts.tile([128, 2], F16, tag="ones")
        nc.vector.memset(ones_sb, 1.0)
        ones32 = consts.tile([1, 2], F32, tag="ones32")
        nc.vector.memset(ones32, 1.0)
        gate_sb = consts.tile([128, T], F32, tag="gate")
        e_sb = consts.tile([128, T], F32, tag="e")

        gelu = mybir.ActivationFunctionType.Gelu
        expf = mybir.ActivationFunctionType.Exp

        unroll = UNROLL if (reps > 1 and reps % UNROLL == 0) else 1
        loop_cm = (tc.For_i(0, reps // unroll, 1) if reps > unroll
                   else nullcontext())
        with loop_cm:
          for _u in range(unroll if reps > 1 else 1):
              if ablate == "noA":
                  nc.vector.memset(gate_sb, 0.125)
                  nc.scalar.activation(out=e_sb, in_=gate_sb, func=expf,
                                       bias=b2_sb[:, 0:1], scale=1.0)
              if ablate not in ("noC", "dmaonly", "mm1only", "samew",
                                "mm1wide", "dr8"):
                  pp = psp.tile([128, H], F32, tag="pp")
                  pd = psd.tile([128, 2], F32, tag="pd")
                  if DENOM == "gps":
                      den_row = small.tile([1, 128], F32, tag="den_row")
                      nc.gpsimd.memset(den_row, 0.0)

              hp_tiles = {}
              exp_done = 0            # tiles whose e is computed
              pool_done = 0           # supertiles pooled so far

              def do_pool(s_lo, s_hi):
                  """Emit ms + pool matmuls for supertiles [s_lo, s_hi)."""
                  for s4 in range(s_lo, s_hi):
                      hpb = hp_tiles.pop(s4)
                      for j in range(4):
                          t = s4 * 4 + j
                          if t >= TU:
                              continue
                          ms = ms_pool.tile([128, 128], F16, tag="ms")
                          # ms[n, g] = (iota[n,g] == bvrel[n]) * e[n]
                          nc.vector.tensor_scalar(
                              out=ms, in0=io_sb,
                              scalar1=bv_sb[:, t:t + 1],
                              scalar2=e_sb[:, t:t + 1],
                              op0=mybir.AluOpType.is_equal,
                              op1=mybir.AluOpType.mult)
                          nc.tensor.matmul(out=pp, lhsT=ms, rhs=hpb[:, j, :],
                                           start=(t == 0), stop=(t == TU - 1))
                          if DENOM == "gps":
                              rsum = small.tile([1, 128], F32, tag="rsum")
                              nc.gpsimd.tensor_reduce(
                                  out=rsum, in_=ms,
                                  axis=mybir.AxisListType.C,
                                  op=mybir.AluOpType.add)
                              nc.gpsimd.tensor_add(out=den_row, in0=den_row,
                                                   in1=rsum)
                          else:
                              nc.tensor.matmul(out=pd, lhsT=ms, rhs=ones_sb,
                                               start=(t == 0),
                                               stop=(t == TU - 1))

              if ablate == "dr8":
                  w18_sb = consts.tile([128, KC, 512], F8E4, tag="w18")
                  nc.sync.dma_start(out=w18_sb, in_=w18_d.ap())
                  for s in range(S):
                      h8 = ht_pool.tile([128, KC, 512], F8E4, tag="h8")
                      nc.sync.dma_start(out=h8, in_=hx8_d.ap()[s])
                      for nch in range(4):
                          pz = psz.tile([128, H], F32, tag="pz")
                          for kp in (0, 2):
                              nc.tensor.matmul(
                                  out=pz,
                                  lhsT=h8[:, kp:kp + 2,
                                          nch * 128:(nch + 1) * 128],
                                  rhs=w18_sb[:, kp:kp + 2, :],
                                  start=(kp == 0), stop=(kp == 2),
                                  perf_mode=mybir.MatmulPerfMode.DoubleRow)
                  osb0 = small.tile([128, H], F32, tag="osb0")
                  nc.vector.memset(osb0, 0.0)
                  nc.sync.dma_start(out=out_d.ap(), in_=osb0)

              if ablate == "mm1wide":
                  hx2 = hx_d.ap()[0:S - 1].rearrange(
                      "(s2 two) p k n -> s2 p k two n", two=2)
                  for s2 in range(S // 2):
                      ht2 = ht_pool.tile([128, KC, 2, 512], F16, tag="ht")
                      nc.sync.dma_start(out=ht2, in_=hx2[s2])
                      for d in range(KC):
                          pzw = psz.tile([128, 2, 512], F32, tag="pzw")
                          for k in range(KC):
                              nc.tensor.matmul(
                                  out=pzw,
                                  lhsT=w1_sb[k][:, d * 128:(d + 1) * 128],
                                  rhs=ht2[:, k, :, :],
                                  start=(k == 0), stop=(k == KC - 1))
                  osb0 = small.tile([128, H], F32, tag="osb0")
                  nc.vector.memset(osb0, 0.0)
                  nc.sync.dma_start(out=out_d.ap(), in_=osb0)

              for s in range(0 if ablate in ("mm1wide", "dr8") else S):
                  # -- DMA issues (both streams interleaved in queue order) --
                  if ablate not in ("noA",):
                      htb = ht_pool.tile([128, KC, 512], hx_dt, tag="ht")
                      nc.sync.dma_start(out=htb, in_=hx_d.ap()[s])
                      if (mode == "flip" and NA > 0
                              and ablate in ("", "dmaonly")):
                          h8b = ht8_pool.tile([128, KC, 512], F8E4, tag="h8")
                          nc.sync.dma_start(out=h8b, in_=hx8_d.ap()[s])
                  if ablate not in ("noC", "mm1only", "samew"):
                      hpb = hp_pool.tile([128, 4, 512], F16, tag="hp")
                      getattr(nc, HPQ).dma_start(out=hpb, in_=hp_d.ap()[s])
                      hp_tiles[s] = hpb

                  if ablate == "dmaonly":
                      continue

                  # -- trickle pool work for exp-ready supertiles --
                  # (emitted BEFORE phase A so the ms ops precede this
                  # supertile's stt ops in the DVE queue; the PE then finds
                  # the pool matmuls' inputs ready instead of stalling on
                  # the gelu->stt->ms chain)
                  if ablate == "":
                      lim = min(exp_done // 4, s)  # strictly-behind supertiles
                      hi = min(pool_done + POOL_RATE, lim)
                      if hi > pool_done:
                          do_pool(pool_done, hi)
                          pool_done = hi

                  # -- phase A compute for supertile s --
                  if ablate != "noA":
                      if mode == "flip":
                          for nch in range(4):
                              tt = s * 4 + nch
                              if tt >= TU and ablate == "":
                                  continue
                              pz = psz.tile([128, H], F32, tag="pz")
                              nsl = slice(nch * 128, (nch + 1) * 128)
                              zoned = (NA > 0 and ablate == "")
                              if zoned:
                                  # zone A: douts [0, NA), both fp8e4,
                                  # k-paired DoubleRow (K=256 per instr)
                                  for kp in (0, 2):
                                      nc.tensor.matmul(
                                          out=pz[:, 0:NA],
                                          lhsT=h8b[:, kp:kp + 2, nsl],
                                          rhs=w1a8_sb[:, kp:kp + 2, :],
                                          start=(kp == 0), stop=(kp == 2),
                                          perf_mode=(mybir.MatmulPerfMode
                                                     .DoubleRow),
                                          skip_group_check=True)
                                  # zone C: douts [NA, H) in f16
                                  if NA < H:
                                      for k in range(KC):
                                          nc.tensor.matmul(
                                              out=pz[:, NA:H],
                                              lhsT=htb[:, k, nsl],
                                              rhs=w1_sb[k][:, NA:H],
                                              start=(k == 0),
                                              stop=(k == KC - 1),
                                              skip_group_check=True)
                              else:
                                  nseg = H // MM1_STREAM
                                  for g0 in range(nseg):
                                      for k in range(KC):
                                          nc.tensor.matmul(
                                              out=pz[:, g0 * MM1_STREAM:
                                                     (g0 + 1) * MM1_STREAM],
                                              lhsT=(htb[:, 0, 0:128]
                                                    if ablate == "samew"
                                                    else htb[:, k, nsl]),
                                              rhs=w1_sb[k][:, g0 * MM1_STREAM:
                                                           (g0 + 1)
                                                           * MM1_STREAM],
                                              start=(k == 0),
                                              stop=(k == KC - 1),
                                              skip_group_check=(nseg > 1))
                              if ablate in ("mm1only", "samew"):
                                  continue
                              a1 = a1_pool.tile([128, H], F16, tag="a1")
                              nc.scalar.activation(out=a1, in_=pz, func=gelu,
                                                   scale=(1.0 / W1SC if zoned
                                                          else 1.0))
                              if ablate == "nogate":
                                  if nch == 0:
                                      nc.vector.memset(
                                          gate_sb[:, s * 4:(s + 1) * 4], 0.125)
                              else:
                                  scr = gs_pool.tile([128, H], F16, tag="scr")
                                  nc.vector.scalar_tensor_tensor(
                                      out=scr, in0=a1, scalar=1.0, in1=w2r_sb,
                                      op0=mybir.AluOpType.mult,
                                      op1=mybir.AluOpType.mult,
                                      accum_out=gate_sb[:, tt:tt + 1])
                      else:
                          a1s = []
                          for d in range(KC):
                              pz = psz.tile([128, H], F32, tag="pz")
                              for k in range(KC):
                                  nc.tensor.matmul(
                                      out=pz,
                                      lhsT=w1_sb[k][:, d * 128:(d + 1) * 128],
                                      rhs=htb[:, k, :],
                                      start=(k == 0), stop=(k == KC - 1))
                              if ablate in ("mm1only", "samew"):
                                  continue
                              a1 = a1_pool.tile([128, H], F16, tag="a1")
                              nc.scalar.activation(out=a1, in_=pz, func=gelu,
                                                   bias=b1_sb[:, d:d + 1],
                                                   scale=1.0)
                              a1s.append(a1)
                          if ablate in ("mm1only", "samew"):
                              pass
                          elif ablate == "nogate":
                              nc.vector.memset(
                                  gate_sb[:, s * 4:(s + 1) * 4], 0.125)
                          else:
                              pg = psg.tile([128, 2 * KC], F32, tag="pg")
                              for nch in range(4):
                                  for d in range(KC):
                                      nc.tensor.matmul(
                                          out=pg[:, 2 * nch:2 * nch + 2],
                                          lhsT=a1s[d][
                                              :, nch * 128:(nch + 1) * 128],
                                          rhs=w2_sb[:, 2 * d:2 * d + 2],
                                          start=(d == 0), stop=(d == KC - 1))
                              nc.vector.tensor_copy(
                                  out=gate_sb[:, s * 4:(s + 1) * 4],
                                  in_=pg[:, 0:2 * KC:2])

                  # -- exp batch --
                  if ablate in ("", "noC") and s in EXP_AT:
                      t_hi = min((s + 1) * 4, TU)
                      nc.scalar.activation(
                          out=e_sb[:, exp_done:t_hi],
                          in_=gate_sb[:, exp_done:t_hi],
                          func=expf, bias=b2_sb[:, 0:1], scale=1.0)
                      exp_done = t_hi

              # -- tail pools --
              if ablate in ("mm1wide", "dr8"):
                  pass
              elif ablate == "nogate":
                  nc.scalar.activation(out=e_sb, in_=gate_sb, func=expf,
                                       bias=0.0, scale=1.0)
              if ablate not in ("noC", "dmaonly", "mm1only", "samew",
                                "mm1wide", "dr8"):
                  do_pool(pool_done, S)

              osb = (None if ablate in ("mm1wide", "dr8")
                     else small.tile([128, H], F32, tag="osb"))
              if ablate in ("mm1wide", "dr8"):
                  pass
              elif ablate in ("noC", "dmaonly", "mm1only", "samew"):
                  nc.vector.memset(osb, 0.0)
              else:
                  if DENOM == "gps":
                      nc.tensor.matmul(out=pd, lhsT=den_row, rhs=ones32,
                                       start=True, stop=True)
                  dcl = small.tile([128, 1], F32, tag="dcl")
                  nc.vector.tensor_scalar(out=dcl, in0=pd[:, 0:1], scalar1=1e-35,
                                          scalar2=None, op0=mybir.AluOpType.max)
                  rec = small.tile([128, 1], F32, tag="rec")
                  nc.vector.reciprocal(out=rec, in_=dcl)
                  nc.vector.tensor_scalar(out=osb, in0=pp, scalar1=rec[:, 0:1],
                                          scalar2=None, op0=mybir.AluOpType.mult)
              if ablate not in ("mm1wide", "dr8"):
                  nc.sync.dma_start(out=out_d.ap(), in_=osb)

    nc.compile()
    if int(os.environ.get("AP_LDWCUT", "0")):
        _cut_redundant_ldw(nc)
    return nc


def _cut_redundant_ldw(nc):
    """Delete InstLdweights that reload the exact weights AP the PE array
    already holds (the pool's pd matmul reuses the pp matmul's ms weights;
    walrus never elides the reload). Only loads with no semaphore role are
    removed, so instruction ordering/sync is unaffected."""
    ncut = 0
    for blk in nc.main_func.blocks:
        insns = blk.instructions
        last_w = None
        to_del = []
        for i in insns:
            if i.engine != mybir.EngineType.PE:
                continue
            if isinstance(i, mybir.InstLdweights):
                w = str(i.ins[0])
                if (w == last_w and i.sync_info is None):
                    to_del.append(i)
                else:
                    last_w = w
            elif not isinstance(i, mybir.InstMatmult):
                last_w = None  # drains etc: assume weights clobbered
        for i in to_del:
            insns.remove(i)
        ncut += len(to_del)
    return ncut


_prog_cache: dict = {}


def _get_prog(np_pad: int, mode: str = None, t_used: int = 0):
    if mode is None:
        mode = MODE
    key = (np_pad, mode, EXP_AT, POOL_RATE, t_used)
    if key not in _prog_cache:
        _prog_cache[key] = _build(np_pad, mode=mode, t_used=t_used)
    return _prog_cache[key]


def _prep_in_maps(h, bv, W1, b1, W2, b2, np_pad, mode=None):
    """Shard + pad inputs per core; returns list of per-core input dicts."""
    if mode is None:
        mode = MODE
    T = np_pad // 128
    S = np_pad // 512
    bounds = np.searchsorted(bv, np.arange(0, NUM_GRAPHS + 1, G))

    zoned = (mode == "flip" and NA > 0)
    if zoned:
        # permute douts by |W2| ascending; scale W1 so fp8/f16 values are
        # in normal range (undone by the gelu activation's scale=1/W1SC)
        order = np.argsort(np.abs(W2[:, 0]), kind="stable")
        W1u = np.ascontiguousarray(W1[:, order].astype(np.float32) * W1SC)
        W2u = np.ascontiguousarray(W2[order])
    else:
        W1u, W2u = W1.astype(np.float32), W2
    # [k, p, dout]
    w1x = np.ascontiguousarray(W1u.astype(np.float16).reshape(4, 128, H))
    b2t = np.full((128, 1), np.float32(b2.reshape(-1)[0]), np.float32)
    iota = np.ascontiguousarray(
        np.tile(np.arange(128, dtype=np.float32), (128, 1)))
    common = {"w1x": w1x, "b2t": b2t, "iota": iota}
    e4 = mybir.dt.np(F8E4)
    if zoned:
        common["w1a8"] = np.ascontiguousarray(
            np.clip(W1u[:, :NA], -224, 224)
            .astype(e4).reshape(4, 128, NA).transpose(1, 0, 2))
    if os.environ.get("AP_DR8_INPUTS"):
        common["w1x8"] = np.ascontiguousarray(
            W1u.astype(e4).reshape(4, 128, H).transpose(1, 0, 2))
    if mode == "flip":
        common["w2rep"] = np.ascontiguousarray(
            np.tile(W2u[:, 0].astype(np.float16), (128, 1)))
    else:
        common["b1v"] = np.ascontiguousarray(
            b1.astype(np.float32).reshape(4, 128).T)
        w2v = np.zeros((128, 8), np.float16)
        w2v[:, 0::2] = W2[:, 0].astype(np.float16).reshape(4, 128).T
        common["W2v"] = w2v

    in_maps = []
    for c in range(N_CORES):
        n0, n1 = int(bounds[c]), int(bounds[c + 1])
        cnt = n1 - n0
        hpad = np.zeros((np_pad, H), np.float32)
        hpad[:cnt] = h[n0:n1]
        # hp: [S, p, j, d]; node = s*512 + j*128 + p
        hp = np.ascontiguousarray(
            hpad.astype(np.float16).reshape(S, 4, 128, H)
            .transpose(0, 2, 1, 3))
        # hx: [S, p, k, n]; hidden = k*128 + p, node = s*512 + n
        hx_npdt = (mybir.dt.np(F8E3) if HXDT == "f8e3" else np.float16)
        hx = np.ascontiguousarray(
            hpad.astype(hx_npdt).reshape(S, 512, 4, 128)
            .transpose(0, 3, 2, 1))
        bvrel = np.full(np_pad, -1.0, np.float32)
        bvrel[:cnt] = bv[n0:n1].astype(np.float32) - c * G
        bvrel = np.ascontiguousarray(bvrel.reshape(T, 128).T)
        entry = {"hx": hx, "hp": hp, "bvrel": bvrel, **common}
        if zoned or os.environ.get("AP_DR8_INPUTS"):
            entry["hx8"] = np.ascontiguousarray(
                np.clip(hpad, -224, 224)
                .astype(e4).reshape(S, 512, 4, 128).transpose(0, 3, 2, 1))
        in_maps.append(entry)
    return in_maps


def kernel(**inputs) -> np.ndarray:
    h = np.ascontiguousarray(np.asarray(inputs["h"], dtype=np.float32))
    bv = np.asarray(inputs["batch_vec"]).astype(np.int64)
    W1 = np.asarray(inputs["W1"], dtype=np.float32)
    b1 = np.asarray(inputs["b1"], dtype=np.float32)
    W2 = np.asarray(inputs["W2"], dtype=np.float32)
    b2 = np.asarray(inputs["b2"], dtype=np.float32)

    bounds = np.searchsorted(bv, np.arange(0, NUM_GRAPHS + 1, G))
    max_cnt = int(np.diff(bounds).max())
    np_pad = NP_DEFAULT
    if max_cnt > np_pad:  # fallback for unexpected distributions
        np_pad = ((max_cnt + 511) // 512) * 512

    # flip mode folds b1 away (it is zero for this module); fall back to
    # the mm2 structure for nonzero b1.
    mode = MODE
    if mode == "flip" and np.any(b1 != 0):
        mode = "mm2"

    t_used = min((max_cnt + 127) // 128, np_pad // 128)
    nc = _get_prog(np_pad, mode, t_used)
    in_maps = _prep_in_maps(h, bv, W1, b1, W2, b2, np_pad, mode)
    trace = bool(int(os.environ.get("AP_TRACE", "0")))
    res = run_bass_kernel_spmd(nc, in_maps, list(range(N_CORES)), trace=trace)
    global last_results
    last_results = res
    out = np.concatenate([res.results[c]["out"] for c in range(N_CORES)],
                         axis=0).astype(np.float32)
    return out


last_results = None

